# revision 9
# baseline (speedup 1.0000x reference)
"""Differential-attention + GroupNorm Trainium2 kernel, 8-core head-parallel.

Problem (hardcoded):
  q, k: [1, 32, 2048, 64] f32 ; v: [1, 16, 2048, 128] f32
  lambda_q1/k1/q2/k2: [64] f32 ; gn_weight/gn_bias: [2048] f32
  out:  [1, 2048, 2048] f32

Sharding: 2 v-heads (= 4 q/k heads) per core across 8 cores. Each core
computes, for each of its v-heads: ghostmax attention w0 - lambda*w1, the
AV product, and the per-head GroupNorm. Scores and AV run in a transposed
layout (keys on partitions, queries free); the small O^T result is
transposed back on the PE so softmax denominators and GroupNorm apply as
cheap per-partition scalars. Host only reshapes/casts (sharding).

Device inputs per core:
  qt   [2, 64, 4096]  bf16 : per v-head, q0^T || q1^T along free dim
  kt   [2, 64, 4096]  bf16 : k0^T || k1^T
  v    [2, 2048, 128] bf16
  lam  [1, 256]       f32  : lambda_q1 | lambda_k1 | lambda_q2 | lambda_k2
  wq   [2, 128, 16]   f32  : gn_weight per (head, q-tile, q%128)
  bq   [2, 128, 16]   f32  : gn_bias * (1-LAMBDA_INIT), same layout
Output:
  out  [2, 128, 2048] f32  : per head, 16 q-tiles of [128 q, 128 d]
                             at columns [128*tt : 128*(tt+1)]
"""
import math
import os
import numpy as np
import ml_dtypes

import concourse.bass as bass
import concourse.mybir as mybir
import concourse.tile as tile
from concourse import bacc
from concourse.bass_utils import run_bass_kernel_spmd
from concourse.masks import make_identity

F32 = mybir.dt.float32
BF16 = mybir.dt.bfloat16
AF = mybir.ActivationFunctionType
ALU = mybir.AluOpType

S = 2048          # sequence length (keys and queries)
D = 64            # head dim of q/k
DV = 128          # head dim of v
HQ = 16           # number of v-heads
NCORE = 8
VH = HQ // NCORE  # v-heads per core = 2
QP = 1024         # queries per pass
NPASS = S // QP   # 2
NCH = S // 128    # 16 key chunks
NQT = QP // 128   # 8 q-tiles per pass
LAMBDA_INIT = 0.8
EPS = 1e-5
SCALE = 1.0 / math.sqrt(D)

_PROGRAM = None


def _build_program():
    nc = bacc.Bacc("TRN2", target_bir_lowering=False, debug=False,
                   num_devices=NCORE)
    qt_d = nc.dram_tensor("qt", [VH, D, 2 * S], BF16, kind="ExternalInput").ap()
    kt_d = nc.dram_tensor("kt", [VH, D, 2 * S], BF16, kind="ExternalInput").ap()
    v_d = nc.dram_tensor("v", [VH, S, DV], BF16, kind="ExternalInput").ap()
    lam_d = nc.dram_tensor("lam", [1, 4 * D], F32, kind="ExternalInput").ap()
    wq_d = nc.dram_tensor("wq", [VH, 128, NCH], F32, kind="ExternalInput").ap()
    bq_d = nc.dram_tensor("bq", [VH, 128, NCH], F32, kind="ExternalInput").ap()
    out_d = nc.dram_tensor("out", [VH, 128, S], F32, kind="ExternalOutput").ap()

    def mm(out, lhsT, rhs, start, stop, n_split=512):
        n = rhs.shape[-1]
        for j in range(0, n, n_split):
            e = min(j + n_split, n)
            nc.tensor.matmul(out[:, j:e], lhsT, rhs[:, j:e],
                             start=start, stop=stop)

    with tile.TileContext(nc) as tc:
        with tc.tile_pool(name="const", bufs=1) as const, \
             tc.tile_pool(name="inp", bufs=1) as inp, \
             tc.tile_pool(name="acc", bufs=2) as accp, \
             tc.tile_pool(name="ework", bufs=3) as ework, \
             tc.tile_pool(name="work", bufs=1) as work, \
             tc.tile_pool(name="oct", bufs=2) as octp, \
             tc.tile_pool(name="ps", bufs=1, space="PSUM") as ps:

            ones = const.tile([128, 128], BF16)
            nc.gpsimd.memset(ones[:], 1.0)
            ident = const.tile([128, 128], F32, tag="ident")
            make_identity(nc, ident)

            # ---- inputs ----
            qts, kts, vts, wqs, bqs = [], [], [], [], []
            for h in range(VH):
                qt = inp.tile([D, 2 * S], BF16, tag=f"qt{h}")
                kt = inp.tile([D, 2 * S], BF16, tag=f"kt{h}")
                nc.sync.dma_start(qt[:], qt_d[h])
                nc.sync.dma_start(kt[:], kt_d[h])
                qts.append(qt)
                kts.append(kt)
                vrow = []
                for c in range(NCH):
                    vc = inp.tile([128, DV], BF16, tag=f"v{h}_{c}")
                    nc.sync.dma_start(vc[:], v_d[h, c * 128:(c + 1) * 128, :])
                    vrow.append(vc)
                vts.append(vrow)
                wqt = inp.tile([128, NCH], F32, tag=f"wq{h}")
                bqt = inp.tile([128, NCH], F32, tag=f"bq{h}")
                nc.sync.dma_start(wqt[:], wq_d[h])
                nc.sync.dma_start(bqt[:], bq_d[h])
                wqs.append(wqt)
                bqs.append(bqt)

            lam = inp.tile([1, 4 * D], F32, tag="lam")
            nc.sync.dma_start(lam[:], lam_d[:])

            # ---- lambda_full = exp(lq1.lk1) - exp(lq2.lk2) + 0.8 -> [128,1]
            scr = work.tile([1, D], F32, tag="lscr")
            s12 = work.tile([1, 2], F32, tag="ls12")
            nc.vector.tensor_tensor(scr[:], lam[:, 0:D], lam[:, D:2 * D],
                                    ALU.mult)
            nc.vector.tensor_reduce(s12[:, 0:1], scr[:],
                                    mybir.AxisListType.X, ALU.add)
            nc.vector.tensor_tensor(scr[:], lam[:, 2 * D:3 * D],
                                    lam[:, 3 * D:4 * D], ALU.mult)
            nc.vector.tensor_reduce(s12[:, 1:2], scr[:],
                                    mybir.AxisListType.X, ALU.add)
            e12 = work.tile([1, 2], F32, tag="le12")
            nc.scalar.activation(e12[:], s12[:], AF.Exp)
            lamf = work.tile([1, 1], F32, tag="lamf")
            nc.vector.tensor_tensor(lamf[:], e12[:, 0:1], e12[:, 1:2],
                                    ALU.subtract)
            nc.vector.tensor_scalar(lamf[:], lamf[:], LAMBDA_INIT, None, ALU.add)
            # hi/lo bf16 split for an exact fp32 broadcast through the PE
            lhi = work.tile([1, 1], BF16, tag="lhi")
            nc.vector.tensor_copy(lhi[:], lamf[:])
            llo = work.tile([1, 1], F32, tag="llo")
            nc.vector.tensor_tensor(llo[:], lamf[:], lhi[:], ALU.subtract)
            llob = work.tile([1, 1], BF16, tag="llob")
            nc.vector.tensor_copy(llob[:], llo[:])
            lam_ps = ps.tile([128, QP], F32, tag="pa")   # borrow pa-tag banks
            nc.tensor.matmul(lam_ps[:, 0:1], ones[0:1, :], lhi[:],
                             start=True, stop=False)
            nc.tensor.matmul(lam_ps[:, 0:1], ones[0:1, :], llob[:],
                             start=False, stop=True)
            neglamv = const.tile([128, 1], F32, tag="neglamv")
            nc.vector.tensor_scalar(neglamv[:], lam_ps[:, 0:1], -1.0, None,
                                    ALU.mult)

            inv_n = 1.0 / float(S * DV)

            # ---- main per-head pipeline ----
            for h in range(VH):
                oct_t = octp.tile([128, S], F32, tag="oct")
                stats = work.tile([128, NCH + NPASS], F32, tag="stats")
                for qp in range(NPASS):
                    qsl = slice(qp * QP, (qp + 1) * QP)
                    q1sl = slice(2048 + qp * QP, 2048 + (qp + 1) * QP)
                    acc = accp.tile([128, 2 * QP], F32, tag="acc")
                    # seed 1/128: the column sum then carries ghostmax's +1
                    nc.gpsimd.memset(acc[:], 1.0 / 128.0)
                    o0 = ps.tile([128, QP], F32, tag="o0")
                    o1 = ps.tile([128, QP], F32, tag="o1")
                    for c in range(NCH):
                        csl = slice(c * 128, (c + 1) * 128)
                        c1sl = slice(2048 + c * 128, 2048 + (c + 1) * 128)
                        pa = ps.tile([128, QP], F32, tag="pa")
                        pb = ps.tile([128, QP], F32, tag="pb")
                        mm(pa, kts[h][:, csl], qts[h][:, qsl], True, True)
                        mm(pb, kts[h][:, c1sl], qts[h][:, q1sl], True, True)
                        ea = ework.tile([128, QP], BF16, tag="ea")
                        eb = ework.tile([128, QP], BF16, tag="eb")
                        nc.scalar.activation(ea[:], pa[:], AF.Exp, scale=SCALE)
                        nc.scalar.activation(eb[:], pb[:], AF.Exp, scale=SCALE)
                        nc.vector.tensor_tensor(acc[:, 0:QP], acc[:, 0:QP],
                                                ea[:], ALU.add)
                        nc.vector.tensor_tensor(acc[:, QP:2 * QP],
                                                acc[:, QP:2 * QP], eb[:],
                                                ALU.add)
                        mm(o0, vts[h][c][:], ea[:], c == 0, c == NCH - 1)
                        mm(o1, vts[h][c][:], eb[:], c == 0, c == NCH - 1)

                    # per-q-tile denominators: 16 single-column ones-matmuls
                    accb = work.tile([128, 2 * QP], BF16, tag="accb")
                    nc.vector.tensor_copy(accb[:], acc[:])
                    dt = ps.tile([128, 2 * NQT], F32, tag="pa")
                    for t in range(2 * NQT):
                        nc.tensor.matmul(dt[:, t:t + 1],
                                         accb[:, t * 128:(t + 1) * 128],
                                         ones[:, 0:1], start=True, stop=True)
                    rt = work.tile([128, 2 * NQT], F32, tag="rt")
                    nc.vector.reciprocal(rt[:], dt[:])
                    r1l = work.tile([128, NQT], F32, tag="r1l")
                    nc.vector.tensor_scalar(r1l[:], rt[:, NQT:2 * NQT],
                                            neglamv[:], None, ALU.mult)

                    # O^T to SBUF, transpose per tile, combine with r scalars
                    o0s = work.tile([128, QP], F32, tag="o0s")
                    o1s = work.tile([128, QP], F32, tag="o1s")
                    nc.vector.tensor_copy(o0s[:], o0[:])
                    nc.vector.tensor_copy(o1s[:], o1[:])
                    for t in range(NQT):
                        tt = qp * NQT + t
                        tsl = slice(t * 128, (t + 1) * 128)
                        ot0 = ps.tile([128, 128], F32, tag="o0")
                        ot1 = ps.tile([128, 128], F32, tag="o1")
                        nc.tensor.transpose(ot0[:], o0s[:, tsl], ident[:])
                        nc.tensor.transpose(ot1[:], o1s[:, tsl], ident[:])
                        tmp = work.tile([128, 128], F32, tag="tmp")
                        nc.vector.tensor_scalar(tmp[:], ot1[:],
                                                r1l[:, t:t + 1], None, ALU.mult)
                        nc.vector.scalar_tensor_tensor(
                            oct_t[:, tt * 128:(tt + 1) * 128], ot0[:],
                            rt[:, t:t + 1], tmp[:], ALU.mult, ALU.add,
                            accum_out=stats[:, tt:tt + 1])
                    scr2 = work.tile([128, QP], F32, tag="scr2")
                    nc.vector.tensor_tensor(scr2[:], oct_t[:, qsl],
                                            oct_t[:, qsl], ALU.mult)
                    nc.vector.tensor_reduce(stats[:, NCH + qp:NCH + qp + 1],
                                            scr2[:], mybir.AxisListType.X,
                                            ALU.add)

                # ---- GroupNorm ----
                s_all = work.tile([128, 2], F32, tag="s_all")
                nc.vector.tensor_reduce(s_all[:, 0:1], stats[:, 0:NCH],
                                        mybir.AxisListType.X, ALU.add)
                nc.vector.tensor_reduce(s_all[:, 1:2],
                                        stats[:, NCH:NCH + NPASS],
                                        mybir.AxisListType.X, ALU.add)
                tot = work.tile([1, 2], F32, tag="tot")
                nc.gpsimd.tensor_reduce(tot[:], s_all[:],
                                        mybir.AxisListType.C, ALU.add)
                mss = work.tile([1, 2], F32, tag="mss")   # mean, E[x^2]
                nc.vector.tensor_scalar(mss[:], tot[:], inv_n, None, ALU.mult)
                var = work.tile([1, 1], F32, tag="var")
                nc.vector.tensor_tensor(var[:], mss[:, 0:1], mss[:, 0:1],
                                        ALU.mult)
                nc.vector.tensor_tensor(var[:], mss[:, 1:2], var[:],
                                        ALU.subtract)
                nc.vector.tensor_scalar(var[:], var[:], EPS, None, ALU.add)
                lnv = work.tile([1, 1], F32, tag="lnv")
                nc.scalar.activation(lnv[:], var[:], AF.Ln)
                inv = work.tile([1, 1], F32, tag="inv")
                nc.scalar.activation(inv[:], lnv[:], AF.Exp, scale=-0.5)
                inv02 = work.tile([1, 1], F32, tag="inv02")
                nc.vector.tensor_scalar(inv02[:], inv[:], 1.0 - LAMBDA_INIT,
                                        None, ALU.mult)
                # broadcast inv02 and mean to [128,1] via hi/lo PE matmuls
                bco = work.tile([1, 4], BF16, tag="bco")
                blo = work.tile([1, 2], F32, tag="blo")
                nc.vector.tensor_copy(bco[:, 0:1], inv02[:])
                nc.vector.tensor_tensor(blo[:, 0:1], inv02[:], bco[:, 0:1],
                                        ALU.subtract)
                nc.vector.tensor_copy(bco[:, 1:2], blo[:, 0:1])
                nc.vector.tensor_copy(bco[:, 2:3], mss[:, 0:1])
                nc.vector.tensor_tensor(blo[:, 1:2], mss[:, 0:1], bco[:, 2:3],
                                        ALU.subtract)
                nc.vector.tensor_copy(bco[:, 3:4], blo[:, 1:2])
                bc_ps = ps.tile([128, QP], F32, tag="pa")
                nc.tensor.matmul(bc_ps[:, 0:1], ones[0:1, :], bco[:, 0:1],
                                 start=True, stop=False)
                nc.tensor.matmul(bc_ps[:, 0:1], ones[0:1, :], bco[:, 1:2],
                                 start=False, stop=True)
                nc.tensor.matmul(bc_ps[:, 1:2], ones[0:1, :], bco[:, 2:3],
                                 start=True, stop=False)
                nc.tensor.matmul(bc_ps[:, 1:2], ones[0:1, :], bco[:, 3:4],
                                 start=False, stop=True)
                inv02v = work.tile([128, 1], F32, tag="inv02v")
                negmv = work.tile([128, 1], F32, tag="negmv")
                nc.vector.tensor_copy(inv02v[:], bc_ps[:, 0:1])
                nc.vector.tensor_scalar(negmv[:], bc_ps[:, 1:2], -1.0, None,
                                        ALU.mult)

                # A[p,tt] = wq*inv*0.2 ; B[p,tt] = A*(-mean) + bq*0.2
                a16 = work.tile([128, NCH], F32, tag="a16")
                b16 = work.tile([128, NCH], F32, tag="b16")
                nc.vector.tensor_scalar(a16[:], wqs[h][:], inv02v[:], None,
                                        ALU.mult)
                nc.vector.scalar_tensor_tensor(
                    b16[:], a16[:], negmv[:], bqs[h][:], ALU.mult, ALU.add)
                outf = work.tile([128, S], F32, tag="outf")
                for tt in range(NCH):
                    tsl = slice(tt * 128, (tt + 1) * 128)
                    nc.vector.tensor_scalar(outf[:, tsl], oct_t[:, tsl],
                                            a16[:, tt:tt + 1],
                                            b16[:, tt:tt + 1],
                                            ALU.mult, ALU.add)
                nc.sync.dma_start(out_d[h], outf[:])

    nc.finalize()
    return nc


def _get_program():
    global _PROGRAM
    if _PROGRAM is None:
        _PROGRAM = _build_program()
    return _PROGRAM


def _prepare_in_maps(q, k, v, lambda_q1, lambda_k1, lambda_q2, lambda_k2,
                     gn_weight, gn_bias):
    q = np.asarray(q)
    k = np.asarray(k)
    v = np.asarray(v)

    lam = np.concatenate([np.asarray(lambda_q1), np.asarray(lambda_k1),
                          np.asarray(lambda_q2), np.asarray(lambda_k2)]
                         ).astype(np.float32).reshape(1, 4 * D)
    # gn params: channel c = h*128 + s//16 -> value per (head, query s)
    w_hq = np.asarray(gn_weight, dtype=np.float32).reshape(HQ, 128)
    b_hq = np.asarray(gn_bias, dtype=np.float32).reshape(HQ, 128)
    w_q = np.repeat(w_hq, 16, axis=1)                    # [HQ, 2048]
    b_q = np.repeat(b_hq, 16, axis=1) * (1.0 - LAMBDA_INIT)
    # device layout [128, 16]: entry [p, tt] = w_q[h, tt*128 + p]
    w_t = w_q.reshape(HQ, NCH, 128).transpose(0, 2, 1).copy()
    b_t = b_q.reshape(HQ, NCH, 128).transpose(0, 2, 1).copy()

    in_maps = []
    for core in range(NCORE):
        heads = [core * VH + i for i in range(VH)]
        qt = np.empty((VH, D, 2 * S), dtype=ml_dtypes.bfloat16)
        kt = np.empty((VH, D, 2 * S), dtype=ml_dtypes.bfloat16)
        vv = np.empty((VH, S, DV), dtype=ml_dtypes.bfloat16)
        wq16 = np.empty((VH, 128, NCH), dtype=np.float32)
        bq16 = np.empty((VH, 128, NCH), dtype=np.float32)
        for i, hh in enumerate(heads):
            qt[i, :, 0:S] = q[0, 2 * hh].T.astype(ml_dtypes.bfloat16)
            qt[i, :, S:2 * S] = q[0, 2 * hh + 1].T.astype(ml_dtypes.bfloat16)
            kt[i, :, 0:S] = k[0, 2 * hh].T.astype(ml_dtypes.bfloat16)
            kt[i, :, S:2 * S] = k[0, 2 * hh + 1].T.astype(ml_dtypes.bfloat16)
            vv[i] = v[0, hh].astype(ml_dtypes.bfloat16)
            wq16[i] = w_t[hh]
            bq16[i] = b_t[hh]
        in_maps.append({"qt": qt, "kt": kt, "v": vv, "lam": lam,
                        "wq": wq16, "bq": bq16})
    return in_maps


def _assemble(results):
    # out[vh] layout: [128 p, 16 tt, 128 d] -> head output [s=tt*128+p, d]
    out_heads = np.empty((HQ, S, DV), dtype=np.float32)
    for core in range(NCORE):
        o = results[core]["out"]                         # [VH, 128, 2048]
        for i in range(VH):
            oh = np.asarray(o[i]).reshape(128, NCH, DV)
            out_heads[core * VH + i] = oh.transpose(1, 0, 2).reshape(S, DV)
    x = out_heads.reshape(HQ * DV, S)                    # [C, S] row-major
    return np.ascontiguousarray(x.T)[None]               # [1, S, C]


def kernel(**inputs):
    nc = _get_program()
    in_maps = _prepare_in_maps(**inputs)
    res = run_bass_kernel_spmd(nc, in_maps, list(range(NCORE)))
    return _assemble(res.results)


# revision 14
# speedup vs baseline: 1.1531x; 1.1531x over previous
"""Differential-attention + GroupNorm Trainium2 kernel, 8-core head-parallel.

Problem (hardcoded):
  q, k: [1, 32, 2048, 64] f32 ; v: [1, 16, 2048, 128] f32
  lambda_q1/k1/q2/k2: [64] f32 ; gn_weight/gn_bias: [2048] f32
  out:  [1, 2048, 2048] f32

Sharding: 2 v-heads (= 4 q/k heads) per core across 8 cores. Each core
computes, for each of its v-heads: ghostmax attention w0 - lambda*w1, the
AV product, and the per-head GroupNorm. Scores and AV run in a transposed
layout (keys on partitions, queries free); the small O^T result is
transposed back on the PE so softmax denominators and GroupNorm apply as
cheap per-partition scalars. Host only reshapes/casts (sharding).

Device inputs per core:
  qt   [2, 64, 4096]  bf16 : per v-head, q0^T || q1^T along free dim
  kt   [2, 64, 4096]  bf16 : k0^T || k1^T
  v    [2, 2048, 128] bf16
  lam  [1, 256]       f32  : lambda_q1 | lambda_k1 | lambda_q2 | lambda_k2
  wq   [2, 128, 16]   f32  : gn_weight per (head, q-tile, q%128)
  bq   [2, 128, 16]   f32  : gn_bias * (1-LAMBDA_INIT), same layout
Output:
  out  [2, 128, 2048] f32  : per head, 16 q-tiles of [128 q, 128 d]
                             at columns [128*tt : 128*(tt+1)]
"""
import math
import os
import numpy as np
import ml_dtypes

import concourse.bass as bass
import concourse.mybir as mybir
import concourse.tile as tile
from concourse import bacc
from concourse.bass_utils import run_bass_kernel_spmd
from concourse.masks import make_identity

F32 = mybir.dt.float32
FP16 = mybir.dt.float16
BF16 = mybir.dt.bfloat16
AF = mybir.ActivationFunctionType
ALU = mybir.AluOpType

S = 2048          # sequence length (keys and queries)
D = 64            # head dim of q/k
DV = 128          # head dim of v
HQ = 16           # number of v-heads
NCORE = 8
VH = HQ // NCORE  # v-heads per core = 2
QP = 1024         # queries per pass
NPASS = S // QP   # 2
NCH = S // 128    # 16 key chunks
NQT = QP // 128   # 8 q-tiles per pass
LAMBDA_INIT = 0.8
EPS = 1e-5
SCALE = 1.0 / math.sqrt(D)

_PROGRAM = None


def _build_program():
    nc = bacc.Bacc("TRN2", target_bir_lowering=False, debug=False,
                   num_devices=NCORE)
    qt_d = nc.dram_tensor("qt", [VH, D, 2 * S], BF16, kind="ExternalInput").ap()
    kt_d = nc.dram_tensor("kt", [VH, D, 2 * S], BF16, kind="ExternalInput").ap()
    v_d = nc.dram_tensor("v", [VH, S, DV], BF16, kind="ExternalInput").ap()
    lam_d = nc.dram_tensor("lam", [1, 4 * D], F32, kind="ExternalInput").ap()
    wq_d = nc.dram_tensor("wq", [VH, 128, NCH], F32, kind="ExternalInput").ap()
    bq_d = nc.dram_tensor("bq", [VH, 128, NCH], F32, kind="ExternalInput").ap()
    out_d = nc.dram_tensor("out", [VH, 128, S], F32, kind="ExternalOutput").ap()

    def mm(out, lhsT, rhs, start, stop, n_split=512):
        n = rhs.shape[-1]
        for j in range(0, n, n_split):
            e = min(j + n_split, n)
            nc.tensor.matmul(out[:, j:e], lhsT, rhs[:, j:e],
                             start=start, stop=stop)

    with tile.TileContext(nc) as tc:
        with tc.tile_pool(name="const", bufs=1) as const, \
             tc.tile_pool(name="inp", bufs=1) as inp, \
             tc.tile_pool(name="acc", bufs=2) as accp, \
             tc.tile_pool(name="ework", bufs=6) as ework, \
             tc.tile_pool(name="work", bufs=1) as work, \
             tc.tile_pool(name="oct", bufs=2) as octp, \
             tc.tile_pool(name="ps", bufs=1, space="PSUM") as ps:

            ones = const.tile([128, 128], BF16)
            nc.gpsimd.memset(ones[:], 1.0)
            ident = const.tile([128, 128], F32, tag="ident")
            make_identity(nc, ident)

            # ---- inputs ----
            qts, kts, vts, wqs, bqs = [], [], [], [], []
            for h in range(VH):
                qt = inp.tile([D, 2 * S], BF16, tag=f"qt{h}")
                kt = inp.tile([D, 2 * S], BF16, tag=f"kt{h}")
                nc.sync.dma_start(qt[:], qt_d[h])
                nc.sync.dma_start(kt[:], kt_d[h])
                qts.append(qt)
                kts.append(kt)
                vrow = []
                for c in range(NCH):
                    vc = inp.tile([128, DV], BF16, tag=f"v{h}_{c}")
                    nc.sync.dma_start(vc[:], v_d[h, c * 128:(c + 1) * 128, :])
                    vrow.append(vc)
                vts.append(vrow)
                wqt = inp.tile([128, NCH], F32, tag=f"wq{h}")
                bqt = inp.tile([128, NCH], F32, tag=f"bq{h}")
                nc.sync.dma_start(wqt[:], wq_d[h])
                nc.sync.dma_start(bqt[:], bq_d[h])
                wqs.append(wqt)
                bqs.append(bqt)

            lam = inp.tile([1, 4 * D], F32, tag="lam")
            nc.sync.dma_start(lam[:], lam_d[:])

            # ---- lambda_full = exp(lq1.lk1) - exp(lq2.lk2) + 0.8 -> [128,1]
            scr = work.tile([1, D], F32, tag="lscr")
            s12 = work.tile([1, 2], F32, tag="ls12")
            nc.vector.tensor_tensor(scr[:], lam[:, 0:D], lam[:, D:2 * D],
                                    ALU.mult)
            nc.vector.tensor_reduce(s12[:, 0:1], scr[:],
                                    mybir.AxisListType.X, ALU.add)
            nc.vector.tensor_tensor(scr[:], lam[:, 2 * D:3 * D],
                                    lam[:, 3 * D:4 * D], ALU.mult)
            nc.vector.tensor_reduce(s12[:, 1:2], scr[:],
                                    mybir.AxisListType.X, ALU.add)
            e12 = work.tile([1, 2], F32, tag="le12")
            nc.scalar.activation(e12[:], s12[:], AF.Exp)
            lamf = work.tile([1, 1], F32, tag="lamf")
            nc.vector.tensor_tensor(lamf[:], e12[:, 0:1], e12[:, 1:2],
                                    ALU.subtract)
            nc.vector.tensor_scalar(lamf[:], lamf[:], LAMBDA_INIT, None, ALU.add)
            # hi/lo bf16 split for an exact fp32 broadcast through the PE
            lhi = work.tile([1, 1], BF16, tag="lhi")
            nc.vector.tensor_copy(lhi[:], lamf[:])
            llo = work.tile([1, 1], F32, tag="llo")
            nc.vector.tensor_tensor(llo[:], lamf[:], lhi[:], ALU.subtract)
            llob = work.tile([1, 1], BF16, tag="llob")
            nc.vector.tensor_copy(llob[:], llo[:])
            lam_ps = ps.tile([128, QP], F32, tag="pa")   # borrow pa-tag banks
            for _w in range(20):
                nc.tensor.matmul(lam_ps[:, 0:512], ones[:], ones[:].broadcast_to([128, 512]) if False else ones[:, 0:128].rearrange("p d -> p d"), start=True, stop=True) if False else None
            # warm-up matmuls: keep PE busy ~4us so HAM reaches full clock
            wsc = const.tile([128, 512], BF16, tag="wsc")
            nc.gpsimd.memset(wsc[:], 0.5)
            for _w in range(10):
                nc.tensor.matmul(lam_ps[:, 0:512], ones[:], wsc[:],
                                 start=True, stop=True)
            nc.tensor.matmul(lam_ps[:, 0:1], ones[0:1, :], lhi[:],
                             start=True, stop=False)
            nc.tensor.matmul(lam_ps[:, 0:1], ones[0:1, :], llob[:],
                             start=False, stop=True)
            neglamv = const.tile([128, 1], F32, tag="neglamv")
            nc.vector.tensor_scalar(neglamv[:], lam_ps[:, 0:1], -1.0, None,
                                    ALU.mult)

            inv_n = 1.0 / float(S * DV)
            means2 = work.tile([1, VH], F32, tag="means2")
            vars2 = work.tile([1, VH], F32, tag="vars2")
            octs = []

            # ---- main per-head pipeline ----
            for h in range(VH):
                oct_t = octp.tile([128, S], F32, tag="oct")
                stats = work.tile([128, 2 * NPASS], F32, tag="stats")
                for qp in range(NPASS):
                    qsl = slice(qp * QP, (qp + 1) * QP)
                    q1sl = slice(2048 + qp * QP, 2048 + (qp + 1) * QP)
                    acc = accp.tile([128, 2 * QP], FP16, tag="acc")
                    # seed 1/128: the column sum then carries ghostmax's +1
                    nc.gpsimd.memset(acc[:], 1.0 / 128.0)
                    o0 = ps.tile([128, QP], F32, tag="o0")
                    o1 = ps.tile([128, QP], F32, tag="o1")
                    for c in range(NCH):
                        csl = slice(c * 128, (c + 1) * 128)
                        c1sl = slice(2048 + c * 128, 2048 + (c + 1) * 128)
                        pa = ps.tile([128, QP], F32, tag="pa")
                        pb = ps.tile([128, QP], F32, tag="pb")
                        mm(pa, kts[h][:, csl], qts[h][:, qsl], True, True)
                        mm(pb, kts[h][:, c1sl], qts[h][:, q1sl], True, True)
                        ea = ework.tile([128, QP], BF16, tag="ea")
                        eb = ework.tile([128, QP], BF16, tag="eb")
                        nc.scalar.activation(ea[:], pa[:], AF.Exp, scale=SCALE)
                        nc.scalar.activation(eb[:], pb[:], AF.Exp, scale=SCALE)
                        nc.vector.tensor_tensor(acc[:, 0:QP], acc[:, 0:QP],
                                                ea[:], ALU.add)
                        eb_eng = nc.gpsimd if c % 2 == 0 else nc.vector
                        eb_eng.tensor_tensor(acc[:, QP:2 * QP],
                                             acc[:, QP:2 * QP], eb[:],
                                             ALU.add)
                        mm(o0, vts[h][c][:], ea[:], c == 0, c == NCH - 1)
                        mm(o1, vts[h][c][:], eb[:], c == 0, c == NCH - 1)

                    # per-q-tile denominators: 16 single-column ones-matmuls
                    accb = work.tile([128, 2 * QP], BF16, tag="accb")
                    nc.vector.tensor_copy(accb[:], acc[:])
                    dt = ps.tile([128, 2 * NQT], F32, tag="pb")
                    for t in range(2 * NQT):
                        nc.tensor.matmul(dt[:, t:t + 1],
                                         accb[:, t * 128:(t + 1) * 128],
                                         ones[:, 0:1], start=True, stop=True)
                    rt = work.tile([128, 2 * NQT], F32, tag="rt")
                    nc.vector.reciprocal(rt[:], dt[:])
                    r1l = work.tile([128, NQT], F32, tag="r1l")
                    nc.vector.tensor_scalar(r1l[:], rt[:, NQT:2 * NQT],
                                            neglamv[:], None, ALU.mult)
                    # broadcast r columns across each 128-wide d block
                    r0q = work.tile([128, QP], F32, tag="r0q")
                    r1q = work.tile([128, QP], F32, tag="r1q")
                    nc.vector.tensor_copy(
                        r0q[:].rearrange("p (t d) -> p t d", t=NQT),
                        rt[:, 0:NQT].broadcast_to([128, NQT, 128]))
                    nc.vector.tensor_copy(
                        r1q[:].rearrange("p (t d) -> p t d", t=NQT),
                        r1l[:].broadcast_to([128, NQT, 128]))

                    # O^T to SBUF, transpose tiles into psum regions, combine
                    o0s = work.tile([128, QP], F32, tag="o0s")
                    o1s = work.tile([128, QP], F32, tag="o1s")
                    nc.vector.tensor_copy(o0s[:], o0[:])
                    nc.vector.tensor_copy(o1s[:], o1[:])
                    ot0r = ps.tile([128, QP], F32, tag="o0")
                    ot1r = ps.tile([128, QP], F32, tag="o1")
                    for t in range(NQT):
                        tsl = slice(t * 128, (t + 1) * 128)
                        nc.tensor.transpose(ot0r[:, tsl], o0s[:, tsl], ident[:])
                        nc.tensor.transpose(ot1r[:, tsl], o1s[:, tsl], ident[:])
                    t0q = work.tile([128, QP], F32, tag="t0q")
                    t1q = work.tile([128, QP], F32, tag="t1q")
                    nc.vector.tensor_tensor(t0q[:], ot0r[:], r0q[:], ALU.mult)
                    nc.vector.tensor_tensor(t1q[:], ot1r[:], r1q[:], ALU.mult)
                    nc.vector.tensor_tensor(oct_t[:, qsl], t0q[:], t1q[:],
                                            ALU.add)
                    nc.vector.tensor_reduce(stats[:, qp:qp + 1],
                                            oct_t[:, qsl],
                                            mybir.AxisListType.X, ALU.add)
                    scr2 = work.tile([128, QP], F32, tag="scr2")
                    nc.vector.tensor_tensor(scr2[:], oct_t[:, qsl],
                                            oct_t[:, qsl], ALU.mult)
                    nc.vector.tensor_reduce(stats[:, NPASS + qp:NPASS + qp + 1],
                                            scr2[:], mybir.AxisListType.X,
                                            ALU.add)

                # ---- GroupNorm stats (inline, cheap) ----
                octs.append(oct_t)
                s_all = work.tile([128, 2], F32, tag="s_all")
                nc.vector.tensor_reduce(s_all[:, 0:1], stats[:, 0:NPASS],
                                        mybir.AxisListType.X, ALU.add)
                nc.vector.tensor_reduce(s_all[:, 1:2],
                                        stats[:, NPASS:2 * NPASS],
                                        mybir.AxisListType.X, ALU.add)
                tot = work.tile([1, 2], F32, tag="tot")
                nc.gpsimd.tensor_reduce(tot[:], s_all[:],
                                        mybir.AxisListType.C, ALU.add)
                mss = work.tile([1, 2], F32, tag="mss")   # mean, E[x^2]
                nc.vector.tensor_scalar(mss[:], tot[:], inv_n, None, ALU.mult)
                nc.vector.tensor_copy(means2[:, h:h + 1], mss[:, 0:1])
                var = work.tile([1, 1], F32, tag="var")
                nc.vector.tensor_tensor(var[:], mss[:, 0:1], mss[:, 0:1],
                                        ALU.mult)
                nc.vector.tensor_tensor(var[:], mss[:, 1:2], var[:],
                                        ALU.subtract)
                nc.vector.tensor_scalar(vars2[:, h:h + 1], var[:], EPS, None,
                                        ALU.add)

            # ---- deferred GroupNorm apply (one ln/exp table switch) ----
            lnv = work.tile([1, VH], F32, tag="lnv")
            nc.scalar.activation(lnv[:], vars2[:], AF.Ln)
            invs = work.tile([1, VH], F32, tag="invs")
            nc.scalar.activation(invs[:], lnv[:], AF.Exp, scale=-0.5)
            for h in range(VH):
                inv02 = work.tile([1, 1], F32, tag="inv02")
                nc.vector.tensor_scalar(inv02[:], invs[:, h:h + 1],
                                        1.0 - LAMBDA_INIT, None, ALU.mult)
                # broadcast inv02 and mean to [128,1] via hi/lo PE matmuls
                bco = work.tile([1, 4], BF16, tag="bco")
                blo = work.tile([1, 2], F32, tag="blo")
                nc.vector.tensor_copy(bco[:, 0:1], inv02[:])
                nc.vector.tensor_tensor(blo[:, 0:1], inv02[:], bco[:, 0:1],
                                        ALU.subtract)
                nc.vector.tensor_copy(bco[:, 1:2], blo[:, 0:1])
                nc.vector.tensor_copy(bco[:, 2:3], means2[:, h:h + 1])
                nc.vector.tensor_tensor(blo[:, 1:2], means2[:, h:h + 1],
                                        bco[:, 2:3], ALU.subtract)
                nc.vector.tensor_copy(bco[:, 3:4], blo[:, 1:2])
                bc_ps = ps.tile([128, QP], F32, tag="pa")
                nc.tensor.matmul(bc_ps[:, 0:1], ones[0:1, :], bco[:, 0:1],
                                 start=True, stop=False)
                nc.tensor.matmul(bc_ps[:, 0:1], ones[0:1, :], bco[:, 1:2],
                                 start=False, stop=True)
                nc.tensor.matmul(bc_ps[:, 1:2], ones[0:1, :], bco[:, 2:3],
                                 start=True, stop=False)
                nc.tensor.matmul(bc_ps[:, 1:2], ones[0:1, :], bco[:, 3:4],
                                 start=False, stop=True)
                inv02v = work.tile([128, 1], F32, tag="inv02v")
                negmv = work.tile([128, 1], F32, tag="negmv")
                nc.vector.tensor_copy(inv02v[:], bc_ps[:, 0:1])
                nc.vector.tensor_scalar(negmv[:], bc_ps[:, 1:2], -1.0, None,
                                        ALU.mult)

                # A[p,tt] = wq*inv*0.2 ; B[p,tt] = A*(-mean) + bq*0.2
                a16 = work.tile([128, NCH], F32, tag="a16")
                b16 = work.tile([128, NCH], F32, tag="b16")
                nc.vector.tensor_scalar(a16[:], wqs[h][:], inv02v[:], None,
                                        ALU.mult)
                nc.vector.scalar_tensor_tensor(
                    b16[:], a16[:], negmv[:], bqs[h][:], ALU.mult, ALU.add)
                outf = work.tile([128, S], F32, tag="outf")
                for tt in range(NCH):
                    tsl = slice(tt * 128, (tt + 1) * 128)
                    nc.vector.tensor_scalar(outf[:, tsl], octs[h][:, tsl],
                                            a16[:, tt:tt + 1],
                                            b16[:, tt:tt + 1],
                                            ALU.mult, ALU.add)
                nc.sync.dma_start(out_d[h], outf[:])

    nc.finalize()
    return nc


def _get_program():
    global _PROGRAM
    if _PROGRAM is None:
        _PROGRAM = _build_program()
    return _PROGRAM


def _prepare_in_maps(q, k, v, lambda_q1, lambda_k1, lambda_q2, lambda_k2,
                     gn_weight, gn_bias):
    q = np.asarray(q)
    k = np.asarray(k)
    v = np.asarray(v)

    lam = np.concatenate([np.asarray(lambda_q1), np.asarray(lambda_k1),
                          np.asarray(lambda_q2), np.asarray(lambda_k2)]
                         ).astype(np.float32).reshape(1, 4 * D)
    # gn params: channel c = h*128 + s//16 -> value per (head, query s)
    w_hq = np.asarray(gn_weight, dtype=np.float32).reshape(HQ, 128)
    b_hq = np.asarray(gn_bias, dtype=np.float32).reshape(HQ, 128)
    w_q = np.repeat(w_hq, 16, axis=1)                    # [HQ, 2048]
    b_q = np.repeat(b_hq, 16, axis=1) * (1.0 - LAMBDA_INIT)
    # device layout [128, 16]: entry [p, tt] = w_q[h, tt*128 + p]
    w_t = w_q.reshape(HQ, NCH, 128).transpose(0, 2, 1).copy()
    b_t = b_q.reshape(HQ, NCH, 128).transpose(0, 2, 1).copy()

    in_maps = []
    for core in range(NCORE):
        heads = [core * VH + i for i in range(VH)]
        qt = np.empty((VH, D, 2 * S), dtype=ml_dtypes.bfloat16)
        kt = np.empty((VH, D, 2 * S), dtype=ml_dtypes.bfloat16)
        vv = np.empty((VH, S, DV), dtype=ml_dtypes.bfloat16)
        wq16 = np.empty((VH, 128, NCH), dtype=np.float32)
        bq16 = np.empty((VH, 128, NCH), dtype=np.float32)
        for i, hh in enumerate(heads):
            qt[i, :, 0:S] = q[0, 2 * hh].T.astype(ml_dtypes.bfloat16)
            qt[i, :, S:2 * S] = q[0, 2 * hh + 1].T.astype(ml_dtypes.bfloat16)
            kt[i, :, 0:S] = k[0, 2 * hh].T.astype(ml_dtypes.bfloat16)
            kt[i, :, S:2 * S] = k[0, 2 * hh + 1].T.astype(ml_dtypes.bfloat16)
            vv[i] = v[0, hh].astype(ml_dtypes.bfloat16)
            wq16[i] = w_t[hh]
            bq16[i] = b_t[hh]
        in_maps.append({"qt": qt, "kt": kt, "v": vv, "lam": lam,
                        "wq": wq16, "bq": bq16})
    return in_maps


def _assemble(results):
    # out[vh] layout: [128 p, 16 tt, 128 d] -> head output [s=tt*128+p, d]
    out_heads = np.empty((HQ, S, DV), dtype=np.float32)
    for core in range(NCORE):
        o = results[core]["out"]                         # [VH, 128, 2048]
        for i in range(VH):
            oh = np.asarray(o[i]).reshape(128, NCH, DV)
            out_heads[core * VH + i] = oh.transpose(1, 0, 2).reshape(S, DV)
    x = out_heads.reshape(HQ * DV, S)                    # [C, S] row-major
    return np.ascontiguousarray(x.T)[None]               # [1, S, C]


def kernel(**inputs):
    nc = _get_program()
    in_maps = _prepare_in_maps(**inputs)
    res = run_bass_kernel_spmd(nc, in_maps, list(range(NCORE)))
    return _assemble(res.results)


# revision 19
# speedup vs baseline: 1.2506x; 1.0846x over previous
"""Differential-attention + GroupNorm Trainium2 kernel, 8-core head-parallel.

Problem (hardcoded):
  q, k: [1, 32, 2048, 64] f32 ; v: [1, 16, 2048, 128] f32
  lambda_q1/k1/q2/k2: [64] f32 ; gn_weight/gn_bias: [2048] f32
  out:  [1, 2048, 2048] f32

Sharding: 2 v-heads (= 4 q/k heads) per core across 8 cores. Each core
computes, for each of its v-heads: ghostmax attention w0 - lambda*w1, the
AV product, and the per-head GroupNorm. Scores and AV run in a transposed
layout (keys on partitions, queries free); the small O^T result is
transposed back on the PE so softmax denominators and GroupNorm apply as
cheap per-partition scalars. Host only reshapes/casts (sharding).

Device inputs per core:
  qt   [2, 64, 4096]  bf16 : per v-head, q0^T || q1^T along free dim
  kt   [2, 64, 4096]  bf16 : k0^T || k1^T
  v    [2, 2048, 128] bf16
  lam  [1, 256]       f32  : lambda_q1 | lambda_k1 | lambda_q2 | lambda_k2
  wq   [2, 128, 16]   f32  : gn_weight per (head, q-tile, q%128)
  bq   [2, 128, 16]   f32  : gn_bias * (1-LAMBDA_INIT), same layout
Output:
  out  [2, 128, 2048] f32  : per head, 16 q-tiles of [128 q, 128 d]
                             at columns [128*tt : 128*(tt+1)]
"""
import math
import os
import numpy as np
import ml_dtypes

import concourse.bass as bass
import concourse.mybir as mybir
import concourse.tile as tile
from concourse import bacc
from concourse.bass_utils import run_bass_kernel_spmd
from concourse.masks import make_identity

F32 = mybir.dt.float32
FP16 = mybir.dt.float16
BF16 = mybir.dt.bfloat16
AF = mybir.ActivationFunctionType
ALU = mybir.AluOpType

S = 2048          # sequence length (keys and queries)
D = 64            # head dim of q/k
DV = 128          # head dim of v
HQ = 16           # number of v-heads
NCORE = 8
VH = HQ // NCORE  # v-heads per core = 2
QP = 512          # queries per pass
NPASS = S // QP   # 2
NCH = S // 128    # 16 key chunks
NQT = QP // 128   # 8 q-tiles per pass
LAMBDA_INIT = 0.8
EPS = 1e-5
SCALE = 1.0 / math.sqrt(D)

_PROGRAM = None


def _build_program():
    nc = bacc.Bacc("TRN2", target_bir_lowering=False, debug=False,
                   num_devices=NCORE)
    qt_d = nc.dram_tensor("qt", [VH, D, 2 * S], BF16, kind="ExternalInput").ap()
    kt_d = nc.dram_tensor("kt", [VH, D, 2 * S], BF16, kind="ExternalInput").ap()
    v_d = nc.dram_tensor("v", [VH, S, DV], BF16, kind="ExternalInput").ap()
    lam_d = nc.dram_tensor("lam", [1, 4 * D], F32, kind="ExternalInput").ap()
    wq_d = nc.dram_tensor("wq", [VH, 128, NCH], F32, kind="ExternalInput").ap()
    bq_d = nc.dram_tensor("bq", [VH, 128, NCH], F32, kind="ExternalInput").ap()
    out_d = nc.dram_tensor("out", [VH, 128, S], F32, kind="ExternalOutput").ap()

    def mm(out, lhsT, rhs, start, stop, n_split=512):
        n = rhs.shape[-1]
        for j in range(0, n, n_split):
            e = min(j + n_split, n)
            nc.tensor.matmul(out[:, j:e], lhsT, rhs[:, j:e],
                             start=start, stop=stop)

    with tile.TileContext(nc) as tc:
        with tc.tile_pool(name="const", bufs=1) as const, \
             tc.tile_pool(name="inp", bufs=1) as inp, \
             tc.tile_pool(name="acc", bufs=2) as accp, \
             tc.tile_pool(name="ework", bufs=8) as ework, \
             tc.tile_pool(name="work", bufs=1) as work, \
             tc.tile_pool(name="oct", bufs=2) as octp, \
             tc.tile_pool(name="ps", bufs=2, space="PSUM") as ps:

            ones = const.tile([128, 128], BF16)
            nc.gpsimd.memset(ones[:], 1.0)
            ident = const.tile([128, 128], F32, tag="ident")
            make_identity(nc, ident)

            # ---- inputs ----
            qts, kts, vts, wqs, bqs = [], [], [], [], []
            for h in range(VH):
                qt = inp.tile([D, 2 * S], BF16, tag=f"qt{h}")
                kt = inp.tile([D, 2 * S], BF16, tag=f"kt{h}")
                nc.sync.dma_start(qt[:], qt_d[h])
                nc.sync.dma_start(kt[:], kt_d[h])
                qts.append(qt)
                kts.append(kt)
                vrow = []
                for c in range(NCH):
                    vc = inp.tile([128, DV], BF16, tag=f"v{h}_{c}")
                    nc.sync.dma_start(vc[:], v_d[h, c * 128:(c + 1) * 128, :])
                    vrow.append(vc)
                vts.append(vrow)
                wqt = inp.tile([128, NCH], F32, tag=f"wq{h}")
                bqt = inp.tile([128, NCH], F32, tag=f"bq{h}")
                nc.sync.dma_start(wqt[:], wq_d[h])
                nc.sync.dma_start(bqt[:], bq_d[h])
                wqs.append(wqt)
                bqs.append(bqt)

            lam = inp.tile([1, 4 * D], F32, tag="lam")
            nc.sync.dma_start(lam[:], lam_d[:])

            # ---- lambda_full = exp(lq1.lk1) - exp(lq2.lk2) + 0.8 -> [128,1]
            scr = work.tile([1, D], F32, tag="lscr")
            s12 = work.tile([1, 2], F32, tag="ls12")
            nc.vector.tensor_tensor(scr[:], lam[:, 0:D], lam[:, D:2 * D],
                                    ALU.mult)
            nc.vector.tensor_reduce(s12[:, 0:1], scr[:],
                                    mybir.AxisListType.X, ALU.add)
            nc.vector.tensor_tensor(scr[:], lam[:, 2 * D:3 * D],
                                    lam[:, 3 * D:4 * D], ALU.mult)
            nc.vector.tensor_reduce(s12[:, 1:2], scr[:],
                                    mybir.AxisListType.X, ALU.add)
            e12 = work.tile([1, 2], F32, tag="le12")
            nc.scalar.activation(e12[:], s12[:], AF.Exp)
            lamf = work.tile([1, 1], F32, tag="lamf")
            nc.vector.tensor_tensor(lamf[:], e12[:, 0:1], e12[:, 1:2],
                                    ALU.subtract)
            nc.vector.tensor_scalar(lamf[:], lamf[:], LAMBDA_INIT, None, ALU.add)
            # hi/lo bf16 split for an exact fp32 broadcast through the PE
            lhi = work.tile([1, 1], BF16, tag="lhi")
            nc.vector.tensor_copy(lhi[:], lamf[:])
            llo = work.tile([1, 1], F32, tag="llo")
            nc.vector.tensor_tensor(llo[:], lamf[:], lhi[:], ALU.subtract)
            llob = work.tile([1, 1], BF16, tag="llob")
            nc.vector.tensor_copy(llob[:], llo[:])
            lam_ps = ps.tile([128, QP], F32, tag="pab")  # borrow pab banks
            for _w in range(20):
                nc.tensor.matmul(lam_ps[:, 0:512], ones[:], ones[:].broadcast_to([128, 512]) if False else ones[:, 0:128].rearrange("p d -> p d"), start=True, stop=True) if False else None
            # warm-up matmuls: keep PE busy ~4us so HAM reaches full clock
            wsc = const.tile([128, 512], BF16, tag="wsc")
            nc.gpsimd.memset(wsc[:], 0.5)
            for _w in range(10):
                nc.tensor.matmul(lam_ps[:, 0:512], ones[:], wsc[:],
                                 start=True, stop=True)
            nc.tensor.matmul(lam_ps[:, 0:1], ones[0:1, :], lhi[:],
                             start=True, stop=False)
            nc.tensor.matmul(lam_ps[:, 0:1], ones[0:1, :], llob[:],
                             start=False, stop=True)
            neglamv = const.tile([128, 1], F32, tag="neglamv")
            nc.vector.tensor_scalar(neglamv[:], lam_ps[:, 0:1], -1.0, None,
                                    ALU.mult)

            inv_n = 1.0 / float(S * DV)
            means2 = work.tile([1, VH], F32, tag="means2")
            vars2 = work.tile([1, VH], F32, tag="vars2")
            octs = []

            # ---- main per-head pipeline (epilogues deferred one pass) ----
            def make_epilogue(h, qp, qsl, acc, o0, o1, oct_t, stats):
                def epi():
                    # per-q-tile denominators: 16 single-column ones-matmuls
                    accb = work.tile([128, 2 * QP], BF16, tag="accb")
                    nc.vector.tensor_copy(accb[:], acc[:])
                    dt = ps.tile([128, 2 * NQT], F32, tag="pab")
                    for t in range(2 * NQT):
                        nc.tensor.matmul(dt[:, t:t + 1],
                                         accb[:, t * 128:(t + 1) * 128],
                                         ones[:, 0:1], start=True, stop=True)
                    rt = work.tile([128, 2 * NQT], F32, tag="rt")
                    nc.vector.reciprocal(rt[:], dt[:])
                    r1l = work.tile([128, NQT], F32, tag="r1l")
                    nc.vector.tensor_scalar(r1l[:], rt[:, NQT:2 * NQT],
                                            neglamv[:], None, ALU.mult)
                    r0q = work.tile([128, QP], F32, tag="r0q")
                    r1q = work.tile([128, QP], F32, tag="r1q")
                    nc.vector.tensor_copy(
                        r0q[:].rearrange("p (t d) -> p t d", t=NQT),
                        rt[:, 0:NQT].broadcast_to([128, NQT, 128]))
                    nc.vector.tensor_copy(
                        r1q[:].rearrange("p (t d) -> p t d", t=NQT),
                        r1l[:].broadcast_to([128, NQT, 128]))
                    o0s = work.tile([128, QP], F32, tag="o0s")
                    o1s = work.tile([128, QP], F32, tag="o1s")
                    nc.vector.tensor_copy(o0s[:], o0[:])
                    nc.vector.tensor_copy(o1s[:], o1[:])
                    ot0r = ps.tile([128, QP], F32, tag="o0")
                    ot1r = ps.tile([128, QP], F32, tag="o1")
                    for t in range(NQT):
                        tsl = slice(t * 128, (t + 1) * 128)
                        nc.tensor.transpose(ot0r[:, tsl], o0s[:, tsl], ident[:])
                        nc.tensor.transpose(ot1r[:, tsl], o1s[:, tsl], ident[:])
                    t0q = work.tile([128, QP], F32, tag="t0q")
                    t1q = work.tile([128, QP], F32, tag="t1q")
                    nc.vector.tensor_tensor(t0q[:], ot0r[:], r0q[:], ALU.mult)
                    nc.vector.tensor_tensor(t1q[:], ot1r[:], r1q[:], ALU.mult)
                    nc.vector.tensor_tensor(oct_t[:, qsl], t0q[:], t1q[:],
                                            ALU.add)
                    nc.vector.tensor_reduce(stats[:, qp:qp + 1],
                                            oct_t[:, qsl],
                                            mybir.AxisListType.X, ALU.add)
                    scr2 = work.tile([128, QP], F32, tag="scr2")
                    nc.vector.tensor_tensor(scr2[:], oct_t[:, qsl],
                                            oct_t[:, qsl], ALU.mult)
                    nc.vector.tensor_reduce(
                        stats[:, NPASS + qp:NPASS + qp + 1], scr2[:],
                        mybir.AxisListType.X, ALU.add)
                return epi

            def finish_head(h, oct_t, stats):
                def fin():
                    octs.append(oct_t)
                    s_all = work.tile([128, 2], F32, tag="s_all")
                    nc.vector.tensor_reduce(s_all[:, 0:1], stats[:, 0:NPASS],
                                            mybir.AxisListType.X, ALU.add)
                    nc.vector.tensor_reduce(s_all[:, 1:2],
                                            stats[:, NPASS:2 * NPASS],
                                            mybir.AxisListType.X, ALU.add)
                    tot = work.tile([1, 2], F32, tag="tot")
                    nc.gpsimd.tensor_reduce(tot[:], s_all[:],
                                            mybir.AxisListType.C, ALU.add)
                    mss = work.tile([1, 2], F32, tag="mss")
                    nc.vector.tensor_scalar(mss[:], tot[:], inv_n, None,
                                            ALU.mult)
                    nc.vector.tensor_copy(means2[:, h:h + 1], mss[:, 0:1])
                    var = work.tile([1, 1], F32, tag="var")
                    nc.vector.tensor_tensor(var[:], mss[:, 0:1], mss[:, 0:1],
                                            ALU.mult)
                    nc.vector.tensor_tensor(var[:], mss[:, 1:2], var[:],
                                            ALU.subtract)
                    nc.vector.tensor_scalar(vars2[:, h:h + 1], var[:], EPS,
                                            None, ALU.add)
                return fin

            pending = []
            head_oct = {}
            for h in range(VH):
                oct_t = octp.tile([128, S], F32, tag="oct")
                stats = work.tile([128, 2 * NPASS], F32, tag="stats")
                head_oct[h] = (oct_t, stats)
                for qp in range(NPASS):
                    qsl = slice(qp * QP, (qp + 1) * QP)
                    q1sl = slice(2048 + qp * QP, 2048 + (qp + 1) * QP)
                    acc = accp.tile([128, 2 * QP], FP16, tag="acc")
                    o0 = ps.tile([128, QP], F32, tag="o0")
                    o1 = ps.tile([128, QP], F32, tag="o1")
                    for c in range(NCH):
                        csl = slice(c * 128, (c + 1) * 128)
                        c1sl = slice(2048 + c * 128, 2048 + (c + 1) * 128)
                        pab = ps.tile([128, 2 * QP], F32, tag="pab")
                        mm(pab[:, 0:QP], kts[h][:, csl], qts[h][:, qsl],
                           True, True)
                        mm(pab[:, QP:2 * QP], kts[h][:, c1sl],
                           qts[h][:, q1sl], True, True)
                        eab = ework.tile([128, 2 * QP], BF16, tag="eab")
                        nc.scalar.activation(eab[:], pab[:], AF.Exp,
                                             scale=SCALE)
                        if c == 0:
                            # seed 1/128: column sums carry ghostmax's +1
                            nc.vector.tensor_scalar(acc[:, 0:QP], eab[:, 0:QP],
                                                    1.0 / 128.0, None, ALU.add)
                            nc.gpsimd.tensor_scalar(acc[:, QP:2 * QP],
                                                    eab[:, QP:2 * QP],
                                                    1.0 / 128.0, None, ALU.add)
                        else:
                            nc.vector.tensor_tensor(acc[:, 0:QP], acc[:, 0:QP],
                                                    eab[:, 0:QP], ALU.add)
                            b_eng = nc.gpsimd if c % 5 < 2 else nc.vector
                            b_eng.tensor_tensor(acc[:, QP:2 * QP],
                                                acc[:, QP:2 * QP],
                                                eab[:, QP:2 * QP], ALU.add)
                        mm(o0, vts[h][c][:], eab[:, 0:QP], c == 0,
                           c == NCH - 1)
                        mm(o1, vts[h][c][:], eab[:, QP:2 * QP], c == 0,
                           c == NCH - 1)
                        if c == 1:
                            for f in pending:
                                f()
                            pending = []
                    pending.append(
                        make_epilogue(h, qp, qsl, acc, o0, o1, oct_t, stats))
                    if qp == NPASS - 1:
                        pending.append(finish_head(h, oct_t, stats))
            for f in pending:
                f()
            pending = []

            # ---- deferred GroupNorm apply (one ln/exp table switch) ----
            lnv = work.tile([1, VH], F32, tag="lnv")
            nc.scalar.activation(lnv[:], vars2[:], AF.Ln)
            invs = work.tile([1, VH], F32, tag="invs")
            nc.scalar.activation(invs[:], lnv[:], AF.Exp, scale=-0.5)
            for h in range(VH):
                inv02 = work.tile([1, 1], F32, tag="inv02")
                nc.vector.tensor_scalar(inv02[:], invs[:, h:h + 1],
                                        1.0 - LAMBDA_INIT, None, ALU.mult)
                # broadcast inv02 and mean to [128,1] via hi/lo PE matmuls
                bco = work.tile([1, 4], BF16, tag="bco")
                blo = work.tile([1, 2], F32, tag="blo")
                nc.vector.tensor_copy(bco[:, 0:1], inv02[:])
                nc.vector.tensor_tensor(blo[:, 0:1], inv02[:], bco[:, 0:1],
                                        ALU.subtract)
                nc.vector.tensor_copy(bco[:, 1:2], blo[:, 0:1])
                nc.vector.tensor_copy(bco[:, 2:3], means2[:, h:h + 1])
                nc.vector.tensor_tensor(blo[:, 1:2], means2[:, h:h + 1],
                                        bco[:, 2:3], ALU.subtract)
                nc.vector.tensor_copy(bco[:, 3:4], blo[:, 1:2])
                bc_ps = ps.tile([128, QP], F32, tag="pab")
                nc.tensor.matmul(bc_ps[:, 0:1], ones[0:1, :], bco[:, 0:1],
                                 start=True, stop=False)
                nc.tensor.matmul(bc_ps[:, 0:1], ones[0:1, :], bco[:, 1:2],
                                 start=False, stop=True)
                nc.tensor.matmul(bc_ps[:, 1:2], ones[0:1, :], bco[:, 2:3],
                                 start=True, stop=False)
                nc.tensor.matmul(bc_ps[:, 1:2], ones[0:1, :], bco[:, 3:4],
                                 start=False, stop=True)
                inv02v = work.tile([128, 1], F32, tag="inv02v")
                negmv = work.tile([128, 1], F32, tag="negmv")
                nc.vector.tensor_copy(inv02v[:], bc_ps[:, 0:1])
                nc.vector.tensor_scalar(negmv[:], bc_ps[:, 1:2], -1.0, None,
                                        ALU.mult)

                # A[p,tt] = wq*inv*0.2 ; B[p,tt] = A*(-mean) + bq*0.2
                a16 = work.tile([128, NCH], F32, tag="a16")
                b16 = work.tile([128, NCH], F32, tag="b16")
                nc.vector.tensor_scalar(a16[:], wqs[h][:], inv02v[:], None,
                                        ALU.mult)
                nc.vector.scalar_tensor_tensor(
                    b16[:], a16[:], negmv[:], bqs[h][:], ALU.mult, ALU.add)
                outf = work.tile([128, S], F32, tag="outf")
                for tt in range(NCH):
                    tsl = slice(tt * 128, (tt + 1) * 128)
                    nc.vector.tensor_scalar(outf[:, tsl], octs[h][:, tsl],
                                            a16[:, tt:tt + 1],
                                            b16[:, tt:tt + 1],
                                            ALU.mult, ALU.add)
                nc.sync.dma_start(out_d[h], outf[:])

    nc.finalize()
    return nc


def _get_program():
    global _PROGRAM
    if _PROGRAM is None:
        _PROGRAM = _build_program()
    return _PROGRAM


def _prepare_in_maps(q, k, v, lambda_q1, lambda_k1, lambda_q2, lambda_k2,
                     gn_weight, gn_bias):
    q = np.asarray(q)
    k = np.asarray(k)
    v = np.asarray(v)

    lam = np.concatenate([np.asarray(lambda_q1), np.asarray(lambda_k1),
                          np.asarray(lambda_q2), np.asarray(lambda_k2)]
                         ).astype(np.float32).reshape(1, 4 * D)
    # gn params: channel c = h*128 + s//16 -> value per (head, query s)
    w_hq = np.asarray(gn_weight, dtype=np.float32).reshape(HQ, 128)
    b_hq = np.asarray(gn_bias, dtype=np.float32).reshape(HQ, 128)
    w_q = np.repeat(w_hq, 16, axis=1)                    # [HQ, 2048]
    b_q = np.repeat(b_hq, 16, axis=1) * (1.0 - LAMBDA_INIT)
    # device layout [128, 16]: entry [p, tt] = w_q[h, tt*128 + p]
    w_t = w_q.reshape(HQ, NCH, 128).transpose(0, 2, 1).copy()
    b_t = b_q.reshape(HQ, NCH, 128).transpose(0, 2, 1).copy()

    in_maps = []
    for core in range(NCORE):
        heads = [core * VH + i for i in range(VH)]
        qt = np.empty((VH, D, 2 * S), dtype=ml_dtypes.bfloat16)
        kt = np.empty((VH, D, 2 * S), dtype=ml_dtypes.bfloat16)
        vv = np.empty((VH, S, DV), dtype=ml_dtypes.bfloat16)
        wq16 = np.empty((VH, 128, NCH), dtype=np.float32)
        bq16 = np.empty((VH, 128, NCH), dtype=np.float32)
        for i, hh in enumerate(heads):
            qt[i, :, 0:S] = q[0, 2 * hh].T.astype(ml_dtypes.bfloat16)
            qt[i, :, S:2 * S] = q[0, 2 * hh + 1].T.astype(ml_dtypes.bfloat16)
            kt[i, :, 0:S] = k[0, 2 * hh].T.astype(ml_dtypes.bfloat16)
            kt[i, :, S:2 * S] = k[0, 2 * hh + 1].T.astype(ml_dtypes.bfloat16)
            vv[i] = v[0, hh].astype(ml_dtypes.bfloat16)
            wq16[i] = w_t[hh]
            bq16[i] = b_t[hh]
        in_maps.append({"qt": qt, "kt": kt, "v": vv, "lam": lam,
                        "wq": wq16, "bq": bq16})
    return in_maps


def _assemble(results):
    # out[vh] layout: [128 p, 16 tt, 128 d] -> head output [s=tt*128+p, d]
    out_heads = np.empty((HQ, S, DV), dtype=np.float32)
    for core in range(NCORE):
        o = results[core]["out"]                         # [VH, 128, 2048]
        for i in range(VH):
            oh = np.asarray(o[i]).reshape(128, NCH, DV)
            out_heads[core * VH + i] = oh.transpose(1, 0, 2).reshape(S, DV)
    x = out_heads.reshape(HQ * DV, S)                    # [C, S] row-major
    return np.ascontiguousarray(x.T)[None]               # [1, S, C]


def kernel(**inputs):
    nc = _get_program()
    in_maps = _prepare_in_maps(**inputs)
    res = run_bass_kernel_spmd(nc, in_maps, list(range(NCORE)))
    return _assemble(res.results)
